# revision 13
# baseline (speedup 1.0000x reference)
"""CQT magnitude kernel for Trainium2 (8 NeuronCores, Bass/Tile).

Strategy
--------
The CQT C[k, n] = sum_l xpad[n*HOP + l] * kernel[k, l] is regrouped over
128-wide l-chunks: with X128[p, j] = xpad[j*128 + p] (the signal transposed
into a [128, cols] SBUF-resident layout) and HOP = 512 = 4*128,

    C[k, n] = sum_c sum_p kernel[k, c*128 + p] * X128[p, c + 4n]

i.e. for every l-chunk c one PE matmul:  lhsT = kernelT[c] ([128 l, bins]),
rhs = strided view of X128 ([128 l, (frame, track) columns]), accumulated in
PSUM over c.  The filterbank rows 128..251 are zero outside a small center
window (constant-Q support shrinks with frequency), so those bins only get
matmuls for the 48 center chunks.

Sharding: the contraction (l-chunk) axis is split 8 ways.  Each core gets a
pre-transposed, pre-packed slice of the filterbank (host-side numpy packing
keeps the program SPMD-uniform: all per-core variation lives in the input
tensors), computes partial re/im sums for ALL bins/tracks/frames, and the
host sums the 8 partials and takes sqrt(re^2 + im^2).

Matmuls run as float32r (full f32 data, 1 cycle/row at N>=256), N = 258
columns = 129 frames x 2 tracks.
"""

import numpy as np

# ---- problem constants (hardcoded per contract) ----
SR_B, SR_TR, SR_T = 2, 2, 65536        # x shape
NTRACKS = SR_B * SR_TR                 # 4
KBINS = 252
L = 69376                              # filterbank window length
HOP = 512
PCH = 128                              # l-chunk width = contraction size
NCH = L // PCH                         # 542 l-chunks
NF = 1 + SR_T // HOP                   # 129 frames
N2 = 2 * NF                            # 258 matmul columns (2 tracks)
NCORES = 8

M1_C0 = 247                            # first nonzero l-chunk for bins 128..251
M1_NCH = 48                            # number of such chunks (validated in proto)
PER0 = 544 // NCORES                   # 68 m0 slots per core (542 padded to 544)
PER1 = M1_NCH // NCORES                # 6 m1 slots per core
SLOTS = PER0 + PER1                    # 74

M0_COLS = PER0 + 4 * (NF - 1)          # 580 xi columns for the m0 block
M1_COLS = PER1 + 4 * (NF - 1)          # 518 xi columns for the m1 block
XI_COLS = M0_COLS + M1_COLS            # 1098
XPAD_COLS = 1056                       # padded signal columns (134912/128 = 1054, +2 pad)

NGROUPS = 10                           # kt DMA groups: 8 slots apiece (last has 2)
GSIZES = [8] * 9 + [2]

_PROG = None


def _build_program():
    import concourse.bass as bass
    import concourse.mybir as mybir
    from concourse import bacc
    from concourse.tile import TileContext

    f32 = mybir.dt.float32
    f32r = mybir.dt.float32r

    nc = bacc.Bacc(None, name="cqt_spmd")
    # xk0 = [xi block | kt group 0] fused into one tensor so the very first
    # matmul depends on exactly ONE DMA semaphore (the S3_LW matmul struct
    # only carries a single sync-wait command).
    XI_F = XI_COLS * 4                     # 4392 xi f32 columns per partition
    xk0_d = nc.dram_tensor("xk0", [128, XI_F + 2048], f32, kind="ExternalInput")
    kt_d = nc.dram_tensor(
        "kt", [NGROUPS - 1, 128, 8 * 256], f32, kind="ExternalInput"
    )
    out_d = nc.dram_tensor("out", [8, 128, N2], f32, kind="ExternalOutput")

    with TileContext(nc) as tc:
        with (
            tc.tile_pool(name="xkp", bufs=1) as xkp,
            tc.tile_pool(name="ktp", bufs=9) as ktp,
            tc.tile_pool(name="stp", bufs=1) as stp,
            tc.tile_pool(name="accp", bufs=1, space="PSUM") as accp,
        ):
            xk_t = xkp.tile([128, XI_F + 2048], f32r)
            # one big DMA: a single dma_start is split across all 16 SDMA
            # engines internally, and completes on a single semaphore
            nc.gpsimd.dma_start(out=xk_t, in_=xk0_d[:, :].bitcast(f32r))

            accs = [
                accp.tile([128, N2], f32, tag=f"acc{b}", name=f"acc{b}")
                for b in range(8)
            ]

            for g in range(NGROUPS):
                gs = GSIZES[g]
                if g == 0:
                    kt_t = None
                else:
                    kt_t = ktp.tile([128, 8 * 256], f32r, tag="kt")
                    nc.gpsimd.dma_start(
                        out=kt_t[:, : gs * 256],
                        in_=kt_d[g - 1, :, : gs * 256].bitcast(f32r),
                    )
                for si in range(gs):
                    s = 8 * g + si
                    m = 0 if s < PER0 else 1
                    first = s == 0 or s == PER0
                    last = s == PER0 - 1 or s == SLOTS - 1
                    for part in range(2):
                        j0 = (si * 2 + part) * 128
                        lhsT = (
                            xk_t[:, XI_F + j0 : XI_F + j0 + 128]
                            if g == 0
                            else kt_t[:, j0 : j0 + 128]
                        )
                        for tp in range(2):
                            base = (
                                s * 4 if m == 0 else M0_COLS * 4 + (s - PER0) * 4
                            ) + tp * 2
                            rhs = bass.AP(
                                tensor=xk_t.tensor,
                                offset=xk_t.offset + base,
                                ap=[xk_t.ap[0], [16, NF], [1, 2]],
                            )
                            nc.tensor.matmul(
                                accs[m * 4 + part * 2 + tp],
                                lhsT,
                                rhs,
                                start=first,
                                stop=last,
                            )

            stage = stp.tile([128, 8, N2], f32)
            for b in range(8):
                nc.vector.tensor_copy(stage[:, b, :], accs[b])
            nc.gpsimd.dma_start(
                out=out_d[:, :, :].rearrange("b p n -> p b n"), in_=stage
            )
    nc.finalize()  # Bacc: runs compile() (reg alloc, event-sem legalization)
    return nc


def _pack_inputs(x, kr, ki):
    xf = np.ascontiguousarray(np.asarray(x, dtype=np.float32).reshape(NTRACKS, SR_T))
    kr = np.asarray(kr, dtype=np.float32)
    ki = np.asarray(ki, dtype=np.float32)

    # transposed filterbank halves, zero-padded to 128 bins each
    krT0 = np.ascontiguousarray(kr[:128].T)          # [L, 128]
    kiT0 = np.ascontiguousarray(ki[:128].T)

    def padT(mat):
        buf = np.zeros((128, L), np.float32)
        buf[: mat.shape[0]] = mat
        return np.ascontiguousarray(buf.T)

    krT1 = padT(kr[128:])
    kiT1 = padT(ki[128:])

    # signal, padded and transposed: XI_full[p, j, t] = xpad[t, j*128 + p]
    xpad = np.zeros((NTRACKS, XPAD_COLS * PCH), np.float32)
    xpad[:, L // 2 : L // 2 + SR_T] = xf
    XI_full = np.ascontiguousarray(
        xpad.reshape(NTRACKS, XPAD_COLS, PCH).transpose(2, 1, 0)
    )  # [128, 1056, 4]

    in_maps = []
    for q in range(NCORES):
        c0 = q * PER0
        c1 = M1_C0 + q * PER1
        kt_all = np.zeros((80, 2, PCH, 128), np.float32)
        n_real = min(PER0, NCH - c0)
        kt_all[:n_real, 0] = krT0[c0 * 128 : (c0 + n_real) * 128].reshape(
            n_real, 128, 128
        )
        kt_all[:n_real, 1] = kiT0[c0 * 128 : (c0 + n_real) * 128].reshape(
            n_real, 128, 128
        )
        kt_all[PER0:SLOTS, 0] = krT1[c1 * 128 : (c1 + PER1) * 128].reshape(
            PER1, 128, 128
        )
        kt_all[PER0:SLOTS, 1] = kiT1[c1 * 128 : (c1 + PER1) * 128].reshape(
            PER1, 128, 128
        )
        ktg = np.ascontiguousarray(
            kt_all.reshape(NGROUPS, 8, 2, 128, 128)
            .transpose(0, 3, 1, 2, 4)
            .reshape(NGROUPS, 128, 2048)
        )
        xi = np.concatenate(
            [XI_full[:, c0 : c0 + M0_COLS], XI_full[:, c1 : c1 + M1_COLS]],
            axis=1,
        ).reshape(128, XI_COLS * 4)
        xk0 = np.ascontiguousarray(np.concatenate([xi, ktg[0]], axis=1))
        in_maps.append({"kt": np.ascontiguousarray(ktg[1:]), "xk0": xk0})
    return in_maps


def _combine(outs):
    re_acc = np.zeros((KBINS, NTRACKS, NF), np.float32)
    im_acc = np.zeros((KBINS, NTRACKS, NF), np.float32)
    for q in range(NCORES):
        out = outs[q]  # [8, 128, 258]
        for b in range(8):
            m, part, tp = b >> 2, (b >> 1) & 1, b & 1
            arr = out[b].reshape(128, NF, 2)
            rows = slice(0, 128) if m == 0 else slice(128, KBINS)
            nrows = 128 if m == 0 else KBINS - 128
            tgt = re_acc if part == 0 else im_acc
            tgt[rows, tp * 2 : (tp + 1) * 2] += arr[:nrows].transpose(0, 2, 1)
    y = np.sqrt(re_acc**2 + im_acc**2)  # [252, 4, 129]
    return np.ascontiguousarray(
        y.reshape(KBINS, SR_B, SR_TR, NF).transpose(1, 0, 3, 2)
    )


def kernel(x, kr, ki):
    global _PROG
    from concourse.bass_utils import run_bass_kernel_spmd

    if _PROG is None:
        _PROG = _build_program()
    in_maps = _pack_inputs(x, kr, ki)
    res = run_bass_kernel_spmd(_PROG, in_maps, core_ids=list(range(NCORES)))
    outs = [res.results[q]["out"] for q in range(NCORES)]
    return _combine(outs)
